# revision 11
# baseline (speedup 1.0000x reference)
"""TK (Transformer-Kernel) ranking model on 8 Trainium2 NeuronCores.

Data parallel: batch 256 -> 8 cores x 32. Each core runs one fused
Bass/Tile kernel over its 32 batch items (processed as 8 quads of 4,
hardware For_i loop over quads).

Layout: feature-major activations [E(partitions), tokens(free)].
Attention is computed transposed (scores [k_tok(p), q_tok(f)]) so the key
mask + 1/sqrt(dh) fold into a single ACT exp; the softmax denominator and
context are matmuls against the exp matrix; the 1/den lands in the PSUM
evict. All matmul operands are bf16 (fp32 accumulate). LayerNorm affines,
biases and mixers are folded into adjacent matmul weights on the host.
"""

import math
import numpy as np
import ml_dtypes

BF = ml_dtypes.bfloat16

NCORES = 8
B, Q, D, E, FF, L, K, NH = 256, 32, 512, 256, 1024, 2, 11, 8
BC = B // NCORES          # 32 per core
NQUAD = BC // 4
DH = E // NH              # 32
PT = E // 128             # 2
DT = D // 128             # 4
FT = FF // 128            # 8
QP = 4 * Q                # packed query free dim (4 batch x 32 tokens)
LN_EPS = 1e-5

_CACHE = {}


# ----------------------------------------------------------------- host prep
def _pos_enc(dim, length):
    ts_ = np.arange(length, dtype=np.float32)
    n = dim // 2
    inv = np.exp(np.arange(n, dtype=np.float32) * (-math.log(10000.0) / (n - 1)))
    st = ts_[:, None] * inv[None, :]
    return np.concatenate([np.sin(st), np.cos(st)], axis=1).astype(np.float32)


def _fm(x):
    """[.., T, E] -> feature-major [.., PT, 128, T]"""
    xt = np.swapaxes(x, -1, -2)
    shp = xt.shape[:-2] + (PT, 128, xt.shape[-1])
    return np.ascontiguousarray(xt.reshape(shp))


def _prep_consts(inp):
    f32 = np.float32
    ipw = np.asarray(inp["in_proj_w"], f32)
    ipb = np.asarray(inp["in_proj_b"], f32)
    ow = np.asarray(inp["out_w"], f32)
    ob = np.asarray(inp["out_b"], f32)
    g1 = np.asarray(inp["ln1_g"], f32); b1 = np.asarray(inp["ln1_b"], f32)
    g2 = np.asarray(inp["ln2_g"], f32); b2 = np.asarray(inp["ln2_b"], f32)
    f1w = np.asarray(inp["ff1_w"], f32); f1b = np.asarray(inp["ff1_b"], f32)
    f2w = np.asarray(inp["ff2_w"], f32); f2b = np.asarray(inp["ff2_b"], f32)
    mixer = float(np.asarray(inp["mixer"]).reshape(-1)[0])
    ms = float(np.asarray(inp["mixer_stop"]).reshape(-1)[0])
    s1w = np.asarray(inp["sw1_w"], f32); s1b = np.asarray(inp["sw1_b"], f32)
    s2w = np.asarray(inp["sw2_w"], f32); s2b = np.asarray(inp["sw2_b"], f32)
    mu = np.asarray(inp["mu"], f32).reshape(-1)
    sigma = np.asarray(inp["sigma"], f32).reshape(-1)
    kw = np.asarray(inp["kernel_w"], f32).reshape(-1)
    alpha = np.asarray(inp["alpha"], f32).reshape(-1)

    c = {}
    pos = _pos_enc(E, max(Q, D))
    c["pos"] = _fm(pos[None])[0]                                   # [2,128,512]
    c["posq"] = np.ascontiguousarray(
        np.tile(c["pos"][:, :, :Q], (1, 1, 4)))                    # [2,128,128]

    gs = np.ones(E, f32)
    bs = np.zeros(E, f32)
    for l in range(L):
        Wq = ipw[l][:E] * gs[None, :]
        Wk = ipw[l][E:2 * E] * gs[None, :]
        Wv = ipw[l][2 * E:] * gs[None, :]
        bqv = ipb[l][:E] + ipw[l][:E] @ bs
        bv = ipb[l][2 * E:] + ipw[l][2 * E:] @ bs
        c[f"wqk{l}"] = np.ascontiguousarray(
            np.concatenate([Wq.T, Wk.T], 1).reshape(PT, 128, 2 * E)).astype(BF)
        c[f"bq{l}"] = bqv.reshape(PT, 128, 1).astype(f32)
        c[f"wv{l}"] = np.ascontiguousarray(Wv.T.reshape(PT, 128, E)).astype(BF)
        woa = np.concatenate([ow[l].T, np.diag(gs)], 0)            # [512,256]
        c[f"wo{l}"] = np.ascontiguousarray(woa.reshape(2 * PT, 128, E)).astype(BF)
        c[f"bo{l}"] = (ob[l] + bs + ow[l] @ bv).reshape(PT, 128, 1).astype(f32)
        W1 = f1w[l] * g1[l][None, :]
        c[f"w1{l}"] = np.ascontiguousarray(W1.T.reshape(PT, 128, FF)).astype(BF)
        c[f"c1{l}"] = (f1b[l] + f1w[l] @ b1[l]).reshape(FT, 128, 1).astype(f32)
        w2a = np.concatenate([f2w[l].T, np.diag(g1[l])], 0)        # [1280,256]
        c[f"w2{l}"] = np.ascontiguousarray(w2a.reshape(FT + PT, 128, E)).astype(BF)
        c[f"c2{l}"] = (f2b[l] + b1[l]).reshape(PT, 128, 1).astype(f32)
        gs, bs = g2[l], b2[l]

    c["hscale"] = ((1 - mixer) * gs).reshape(PT, 128, 1).astype(f32)
    c["hbias"] = ((1 - mixer) * bs).reshape(PT, 128, 1).astype(f32)
    wza = np.concatenate([(ms * s1w).T, ((1 - ms) * s1w * gs[None, :]).T], 0)
    wz = np.zeros((2 * PT, 128, 128), f32)
    wz[:, :, :100] = wza.reshape(2 * PT, 128, 100)
    c["wz"] = wz.astype(BF)
    bz2 = np.zeros((128, 1), f32)
    bz2[:100, 0] = 2.0 * (s1b + (1 - ms) * (s1w @ bs))
    c["bz2"] = bz2
    s2rep = np.zeros((128, 32), f32)
    s2rep[:100, :] = s2w.reshape(100, 1)
    s2rep[100, :] = 1.0
    c["sw2rep"] = s2rep.astype(BF)
    c["bsw"] = np.full((128, 1), s2b[0], f32)

    c["negmu"] = np.tile(-mu[None, :], (128, 1)).astype(f32)
    c["kw4"] = np.tile(kw[None, :], (4, 1)).astype(f32)
    c["ones_scl"] = np.full((128, 1), 1.0 / E, f32).astype(BF)
    c["ones_col"] = np.ones((128, 1), f32).astype(BF)
    c["ones32"] = np.ones((128, 32), f32).astype(BF)
    c["ones_row"] = np.ones((1, 128), f32).astype(BF)
    c["epsc"] = np.array([[1e-5, 1e-26]], f32).repeat(128, 0)  # [128,2]
    id4 = np.zeros((32, 128), f32)
    for a in range(4):
        id4[np.arange(32), 32 * a + np.arange(32)] = 1.0
    c["id4"] = id4.astype(BF)

    c["_mixer"] = mixer
    c["_aks"] = (-1.0 / (2.0 * sigma ** 2)).astype(f32)
    c["_alpha"] = alpha.astype(f32)
    return c


def _prep_percore(inp, core):
    f32 = np.float32
    sl = slice(core * BC, (core + 1) * BC)
    qe = np.asarray(inp["query_embeddings"], f32)[sl]
    de = np.asarray(inp["document_embeddings"], f32)[sl]
    qm = np.asarray(inp["query_mask"], f32)[sl]
    dm = np.asarray(inp["document_mask"], f32)[sl]
    d = {}
    d["demb"] = _fm(de)                                            # [BC,2,128,512]
    d["qemb"] = _fm(qe)                                            # [BC,2,128,32]
    d["dlogm"] = np.ascontiguousarray(
        ((dm - 1.0) * 1e9).reshape(BC, DT, 128, 1)).astype(f32)
    d["dlogm_row"] = ((dm - 1.0) * 1e9).reshape(BC, 1, D).astype(BF)
    d["qlogm4"] = np.ascontiguousarray(
        np.tile((qm - 1.0) * 1e9, (1, 4)).reshape(BC, 128, 1)).astype(f32)
    qbd = np.zeros((NQUAD, 128, 4), f32)
    for g in range(NQUAD):
        for j in range(4):
            qbd[g, 32 * j:32 * (j + 1), j] = qm[4 * g + j]
    d["qmask_bd"] = np.repeat(qbd, 4, axis=0).astype(BF)           # [BC,128,4]
    return d


# ------------------------------------------------------------------- builder
def _build(consts, debug=False):
    import concourse.bacc as bacc
    import concourse.mybir as mybir
    from concourse.bass import ds
    from concourse.tile import TileContext
    from contextlib import ExitStack

    f32 = mybir.dt.float32
    bf16 = mybir.dt.bfloat16
    AF = mybir.ActivationFunctionType
    AL = mybir.AluOpType

    mixer = consts["_mixer"]
    aks = [float(a) for a in consts["_aks"]]
    alph = [float(a) for a in consts["_alpha"]]
    SCL = 1.0 / math.sqrt(DH)

    nc = bacc.Bacc("TRN2", target_bir_lowering=False, debug=False)

    def din(name, shape, dt=f32):
        return nc.dram_tensor(name, list(shape), dt, kind="ExternalInput")

    demb = din("demb", (BC, PT, 128, D))
    qemb = din("qemb", (BC, PT, 128, Q))
    dlogm = din("dlogm", (BC, DT, 128, 1))
    dlogm_row = din("dlogm_row", (BC, 1, D), bf16)
    qlogm4 = din("qlogm4", (BC, 128, 1))
    qmask_bd = din("qmask_bd", (BC, 128, 4), bf16)
    pos = din("pos", (PT, 128, D))
    posq = din("posq", (PT, 128, QP))
    wqk = [din(f"wqk{l}", (PT, 128, 2 * E), bf16) for l in range(L)]
    bq = [din(f"bq{l}", (PT, 128, 1)) for l in range(L)]
    wv = [din(f"wv{l}", (PT, 128, E), bf16) for l in range(L)]
    wo = [din(f"wo{l}", (2 * PT, 128, E), bf16) for l in range(L)]
    bo = [din(f"bo{l}", (PT, 128, 1)) for l in range(L)]
    w1 = [din(f"w1{l}", (PT, 128, FF), bf16) for l in range(L)]
    c1 = [din(f"c1{l}", (FT, 128, 1)) for l in range(L)]
    w2 = [din(f"w2{l}", (FT + PT, 128, E), bf16) for l in range(L)]
    c2 = [din(f"c2{l}", (PT, 128, 1)) for l in range(L)]
    hscale = din("hscale", (PT, 128, 1))
    hbias = din("hbias", (PT, 128, 1))
    wz = din("wz", (2 * PT, 128, 128), bf16)
    bz2 = din("bz2", (128, 1))
    sw2rep = din("sw2rep", (128, 32), bf16)
    bsw = din("bsw", (128, 1))
    negmu = din("negmu", (128, K))
    kw4 = din("kw4", (4, K))
    ones_scl = din("ones_scl", (128, 1), bf16)
    ones_col = din("ones_col", (128, 1), bf16)
    ones32 = din("ones32", (128, 32), bf16)
    ones_row = din("ones_row", (1, 128), bf16)
    id4 = din("id4", (32, 128), bf16)
    epsc = din("epsc", (128, 2))

    score_out = nc.dram_tensor("score_out", [BC], f32, kind="ExternalOutput")
    sw_out = nc.dram_tensor("sw_out", [BC, D], f32, kind="ExternalOutput")
    dbg = {}
    if debug:
        def dout(name, shape, dt=bf16):
            dbg[name] = nc.dram_tensor(name, list(shape), dt, kind="ExternalOutput")
        dout("dbg_s0", (128, PT, D))
        dout("dbg_u", (128, PT, D))
        dout("dbg_xn1", (128, PT, D))
        dout("dbg_s1", (128, PT, D))
        dout("dbg_den", (128, D), f32)
        dout("dbg_ctx", (128, PT, D))
        dout("dbg_dn", (128, PT, D))
        dout("dbg_qn", (128, PT, QP))
        dout("dbg_cos", (128, D), f32)
        dout("dbg_tnh", (128, D))
        dout("dbg_swb", (128, D), f32)
        dout("dbg_pkq", (128, K), f32)
        dout("dbg_sq", (128, PT, QP))

    with TileContext(nc) as tc, ExitStack() as top:
        wp = top.enter_context(tc.tile_pool(name="wpool", bufs=1))

        def ld2(apdram, dt, name, n, x):
            t = wp.tile([128, n, x], dt, name=name)
            for p in range(n):
                nc.sync.dma_start(t[:, p, :], apdram[p])
            return t

        def ld1(apdram, shape, dt, name):
            t = wp.tile(list(shape), dt, name=name)
            nc.sync.dma_start(t[:], apdram[:])
            return t

        t_pos = ld2(pos, f32, "t_pos", PT, D)
        t_posq = ld2(posq, f32, "t_posq", PT, QP)
        t_wqk = [ld2(wqk[l], bf16, f"t_wqk{l}", PT, 2 * E) for l in range(L)]
        t_bq = [ld2(bq[l], f32, f"t_bq{l}", PT, 1) for l in range(L)]
        t_wv = [ld2(wv[l], bf16, f"t_wv{l}", PT, E) for l in range(L)]
        t_wo = [ld2(wo[l], bf16, f"t_wo{l}", 2 * PT, E) for l in range(L)]
        t_bo = [ld2(bo[l], f32, f"t_bo{l}", PT, 1) for l in range(L)]
        t_w1 = [ld2(w1[l], bf16, f"t_w1{l}", PT, FF) for l in range(L)]
        t_c1 = [ld2(c1[l], f32, f"t_c1{l}", FT, 1) for l in range(L)]
        t_w2 = [ld2(w2[l], bf16, f"t_w2{l}", FT + PT, E) for l in range(L)]
        t_c2 = [ld2(c2[l], f32, f"t_c2{l}", PT, 1) for l in range(L)]
        t_hs = ld2(hscale, f32, "t_hs", PT, 1)
        t_hb = ld2(hbias, f32, "t_hb", PT, 1)
        t_wz = ld2(wz, bf16, "t_wz", 2 * PT, 128)
        t_bz2 = ld1(bz2, (128, 1), f32, "t_bz2")
        t_sw2 = ld1(sw2rep, (128, 32), bf16, "t_sw2")
        t_bsw = ld1(bsw, (128, 1), f32, "t_bsw")
        t_negmu = ld1(negmu, (128, K), f32, "t_negmu")
        t_kw4 = ld1(kw4, (4, K), f32, "t_kw4")
        t_oscl = ld1(ones_scl, (128, 1), bf16, "t_oscl")
        t_ocol = ld1(ones_col, (128, 1), bf16, "t_ocol")
        t_o32 = ld1(ones32, (128, 32), bf16, "t_o32")
        t_orow = ld1(ones_row, (1, 128), bf16, "t_orow")
        t_id4 = ld1(id4, (32, 128), bf16, "t_id4")
        t_eps = ld1(epsc, (128, 2), f32, "t_eps")

        def mm(psum_ap, lhsT, rhs, start, stop, tp=None):
            nc.tensor.matmul(psum_ap, lhsT, rhs, start=start, stop=stop,
                             tile_position=tp)

        with tc.For_i(0, BC, 4) as g, ExitStack() as body:
            w_demb = demb[ds(g, 4)]
            w_qemb = qemb[ds(g, 4)]
            w_dlogm = dlogm[ds(g, 4)]
            w_dlogm_row = dlogm_row[ds(g, 4)]
            w_qlogm4 = qlogm4[ds(g, 4)]
            w_qbd = qmask_bd[ds(g, 4)]
            w_score = score_out[ds(g, 4)]
            w_sw = sw_out[ds(g, 4)]

            iop = body.enter_context(tc.tile_pool(name="io", bufs=1))
            embd, t_dlg, t_qlg = [], [], []
            embqP = iop.tile([128, PT, QP], f32, name="embqP")
            for j in range(4):
                t = iop.tile([128, PT, D], f32, name=f"embd{j}")
                for p in range(PT):
                    nc.sync.dma_start(t[:, p, :], w_demb[j, p])
                embd.append(t)
                for p in range(PT):
                    nc.sync.dma_start(embqP[:, p, 32 * j:32 * (j + 1)],
                                      w_qemb[j, p])
                t = iop.tile([128, DT], f32, name=f"dlg{j}")
                for dt_ in range(DT):
                    nc.sync.dma_start(t[:, dt_:dt_ + 1], w_dlogm[j, dt_])
                t_dlg.append(t)
                t = iop.tile([128, 1], f32, name=f"qlg{j}")
                nc.sync.dma_start(t[:], w_qlogm4[j])
                t_qlg.append(t)
            t_qbd = iop.tile([128, 4], bf16, name="t_qbd")
            nc.sync.dma_start(t_qbd[:], w_qbd[0])

            # streams (bf16, feature-major); query packed 4b along free
            sD = [iop.tile([128, PT, D], bf16, name=f"sD{j}") for j in range(4)]
            sQ = iop.tile([128, PT, QP], bf16, name="sQ")
            with tc.tile_pool(name="prep", bufs=2):
                for j in range(4):
                    for p in range(PT):
                        nc.vector.tensor_add(sD[j][:, p, :], embd[j][:, p, :],
                                             t_pos[:, p, :])
                for p in range(PT):
                    nc.vector.tensor_add(sQ[:, p, :], embqP[:, p, :],
                                         t_posq[:, p, :])
            if debug:
                nc.sync.dma_start(dbg["dbg_s0"][:], sD[0][:])

            for l in range(L):
                with ExitStack() as lay:
                    # ------------ qkv (doc + query) -------------------------
                    sbA = lay.enter_context(tc.tile_pool(name=f"sbA{l}", bufs=1))
                    qfD = [sbA.tile([128, PT, D], bf16, name=f"qfD{l}{j}") for j in range(4)]
                    kfD = [sbA.tile([128, PT, D], bf16, name=f"kfD{l}{j}") for j in range(4)]
                    vD = [sbA.tile([128, DT, E], bf16, name=f"vD{l}{j}") for j in range(4)]
                    qfQ = sbA.tile([128, PT, QP], bf16, name=f"qfQ{l}")
                    kfQ = sbA.tile([128, PT, QP], bf16, name=f"kfQ{l}")
                    vQr = sbA.tile([128, 4, E], bf16, name=f"vQr{l}")
                    with ExitStack() as phA:
                        qkp = phA.enter_context(
                            tc.tile_pool(name=f"qkps{l}", bufs=2, space="PSUM"))
                        psA = phA.enter_context(
                            tc.tile_pool(name=f"psA{l}", bufs=1, space="PSUM"))
                        for j in range(4):
                            for mt in range(4):
                                ps = qkp.tile([128, D], f32, name=f"qk{l}{j}{mt}", tag="qk")
                                for ks in range(PT):
                                    mm(ps[:], t_wqk[l][:, ks, 128 * mt:128 * (mt + 1)],
                                       sD[j][:, ks, :], ks == 0, ks == PT - 1)
                                if mt < PT:
                                    nc.scalar.activation(qfD[j][:, mt, :], ps[:],
                                                         AF.Identity,
                                                         bias=t_bq[l][:, mt, :])
                                else:
                                    nc.scalar.copy(kfD[j][:, mt - PT, :], ps[:])
                            for dt_ in range(DT):
                                ps = qkp.tile([128, E], f32, name=f"vv{l}{j}{dt_}", tag="vv")
                                for ks in range(PT):
                                    mm(ps[:], sD[j][:, ks, 128 * dt_:128 * (dt_ + 1)],
                                       t_wv[l][:, ks, :], ks == 0, ks == PT - 1)
                                nc.scalar.copy(vD[j][:, dt_, :], ps[:])
                        for mt in range(4):
                            ps = qkp.tile([128, QP], f32, name=f"qkq{l}{mt}", tag="vv")
                            for ks in range(PT):
                                mm(ps[:], t_wqk[l][:, ks, 128 * mt:128 * (mt + 1)],
                                   sQ[:, ks, :], ks == 0, ks == PT - 1)
                            if mt < PT:
                                nc.scalar.activation(qfQ[:, mt, :], ps[:], AF.Identity,
                                                     bias=t_bq[l][:, mt, :])
                            else:
                                nc.scalar.copy(kfQ[:, mt - PT, :], ps[:])
                        psvq = psA.tile([32, 4, E], f32, name=f"vq{l}", tag="vq")
                        for j in range(4):
                            for ks in range(PT):
                                mm(psvq[:, j, :], sQ[:, ks, 32 * j:32 * (j + 1)],
                                   t_wv[l][:, ks, :], ks == 0, ks == PT - 1)
                        vq_sb = sbA.tile([32, 4, E], bf16, name=f"vqsb{l}")
                        nc.scalar.copy(vq_sb[:], psvq[:])
                        for half in range(2):
                            psr = psA.tile([128, 2 * E], f32, name=f"vqr{l}{half}", tag="vqrp")
                            mm(psr[:], t_id4[:],
                               vq_sb[:].rearrange("p a e -> p (a e)")[:, 512 * half:512 * (half + 1)],
                               True, True)
                            nc.vector.tensor_copy(
                                vQr[:].rearrange("p a e -> p (a e)")[:, 512 * half:512 * (half + 1)],
                                psr[:])

                    # ------------ attention (doc per-b; query per-b tiny) ---
                    sbB = lay.enter_context(tc.tile_pool(name=f"sbB{l}", bufs=1))
                    ctxD = [sbB.tile([128, PT, D], bf16, name=f"ctxD{l}{j}") for j in range(4)]
                    ctxQ = sbB.tile([128, PT, QP], bf16, name=f"ctxQ{l}")
                    with ExitStack() as phB:
                        psB = phB.enter_context(
                            tc.tile_pool(name=f"psB{l}", bufs=1, space="PSUM"))
                        etp = phB.enter_context(tc.tile_pool(name=f"et{l}", bufs=3))
                        red = phB.enter_context(tc.tile_pool(name=f"red{l}", bufs=2))
                        for j in range(4):
                            psden = [psB.tile([128, D], f32, name=f"den{l}{j}{hq}", tag=f"den{hq}") for hq in range(2)]
                            psctx = [psB.tile([128, D], f32, name=f"ctx{l}{j}{hq}", tag=f"ctxp{hq}") for hq in range(2)]
                            for kt in range(DT):
                                for hq in range(2):
                                    pssc = [psB.tile([128, D], f32, name=f"sc{l}{j}{kt}{hq}{jj}", tag=f"sc{jj}") for jj in range(4)]
                                    for jj in range(4):
                                        mm(pssc[jj][:],
                                           kfD[j][32 * jj:32 * (jj + 1), hq, 128 * kt:128 * (kt + 1)],
                                           qfD[j][32 * jj:32 * (jj + 1), hq, :],
                                           True, True, tp=(32 * jj, 0))
                                    et = etp.tile([128, 4, D], bf16, name=f"et{l}{j}{kt}{hq}", tag="et")
                                    for jj in range(4):
                                        nc.scalar.activation(et[:, jj, :], pssc[jj][:],
                                                             AF.Exp,
                                                             bias=t_dlg[j][:, kt:kt + 1],
                                                             scale=SCL)
                                    for jj in range(4):
                                        mm(psden[hq][32 * jj:32 * (jj + 1), :],
                                           t_o32[:], et[:, jj, :],
                                           kt == 0, kt == DT - 1, tp=(0, 32 * jj))
                                        mm(psctx[hq][32 * jj:32 * (jj + 1), :],
                                           vD[j][:, kt, 128 * hq + 32 * jj:128 * hq + 32 * (jj + 1)],
                                           et[:, jj, :],
                                           kt == 0, kt == DT - 1, tp=(0, 32 * jj))
                            for hq in range(2):
                                rec = red.tile([128, D], f32, name=f"recd{l}{j}{hq}", tag="recd")
                                nc.vector.reciprocal_approx_fast(out=rec[:], in_=psden[hq][:])
                                nc.vector.tensor_mul(ctxD[j][:, hq, :], psctx[hq][:], rec[:])
                            if debug and l == 0 and j == 0:
                                dcp = red.tile([128, D], f32, name="dbgden", tag="recd")
                                nc.vector.tensor_copy(dcp[:], psden[0][:])
                                nc.sync.dma_start(dbg["dbg_den"][:], dcp[:])
                        if debug and l == 0:
                            nc.sync.dma_start(dbg["dbg_ctx"][:], ctxD[0][:])

                        for j in range(4):
                            bsl = slice(32 * j, 32 * (j + 1))
                            psq_sc = [psB.tile([128, Q], f32, name=f"qsc{l}{j}{hq}", tag=f"sc{hq}") for hq in range(2)]
                            psq_dc = [psB.tile([128, 2 * Q], f32, name=f"qdc{l}{j}{hq}", tag=f"sc{2 + hq}") for hq in range(2)]
                            etq = [etp.tile([128, Q], bf16, name=f"etq{l}{j}{hq}", tag="etq") for hq in range(2)]
                            for hq in range(2):
                                for jj in range(4):
                                    rsl = slice(32 * jj, 32 * (jj + 1))
                                    mm(psq_sc[hq][rsl, :], kfQ[rsl, hq, bsl],
                                       qfQ[rsl, hq, bsl], True, True,
                                       tp=(32 * jj, 32 * jj))
                                nc.scalar.activation(etq[hq][:], psq_sc[hq][:], AF.Exp,
                                                     bias=t_qlg[j][:], scale=SCL)
                                for jj in range(4):
                                    rsl = slice(32 * jj, 32 * (jj + 1))
                                    mm(psq_dc[hq][rsl, 0:Q], t_o32[rsl, :],
                                       etq[hq][rsl, :], True, True,
                                       tp=(32 * jj, 32 * jj))
                                    mm(psq_dc[hq][rsl, Q:2 * Q],
                                       vQr[rsl, j, 128 * hq + 32 * jj:128 * hq + 32 * (jj + 1)],
                                       etq[hq][rsl, :], True, True,
                                       tp=(32 * jj, 32 * jj))
                                rec = red.tile([128, Q], f32, name=f"recq{l}{j}{hq}", tag="recq")
                                nc.vector.reciprocal_approx_fast(out=rec[:], in_=psq_dc[hq][:, 0:Q])
                                nc.vector.tensor_mul(ctxQ[:, hq, bsl],
                                                     psq_dc[hq][:, Q:2 * Q], rec[:])

                    # ------------ o-proj + LN1 + ff + LN2 -------------------
                    with ExitStack() as phC:
                        psO = phC.enter_context(tc.tile_pool(name=f"psO{l}", bufs=2, space="PSUM"))
                        psS = phC.enter_context(tc.tile_pool(name=f"psS{l}", bufs=1, space="PSUM"))
                        sbC = phC.enter_context(tc.tile_pool(name=f"sbC{l}", bufs=2))

                        def self_ln(u, TK, tag2):
                            usq = sbC.tile([128, PT, TK], bf16, name=f"usq{l}{tag2}", tag=f"usq{TK}")
                            for p in range(PT):
                                nc.vector.tensor_mul(usq[:, p, :], u[:, p, :], u[:, p, :])
                            st0 = psS.tile([1, TK], f32, name=f"st0{l}{tag2}", tag="st0")
                            st1 = psS.tile([1, TK], f32, name=f"st1{l}{tag2}", tag="st1")
                            for p in range(PT):
                                mm(st0[:], t_oscl[:], u[:, p, :], p == 0, p == PT - 1)
                            for p in range(PT):
                                mm(st1[:], t_oscl[:], usq[:, p, :], p == 0, p == PT - 1)
                            m_sb = sbC.tile([1, TK], f32, name=f"msb{l}{tag2}", tag="msb")
                            nc.vector.tensor_copy(m_sb[:], st0[:])
                            m2 = sbC.tile([1, TK], f32, name=f"m2{l}{tag2}", tag="m2")
                            nc.vector.tensor_mul(m2[:], m_sb[:], m_sb[:])
                            var = sbC.tile([1, TK], f32, name=f"var{l}{tag2}", tag="var")
                            nc.vector.tensor_sub(var[:], st1[:], m2[:])
                            lnv = sbC.tile([1, TK], f32, name=f"lnv{l}{tag2}", tag="lnv")
                            nc.scalar.activation(lnv[:], var[:], AF.Ln,
                                                 bias=t_eps[0:1, 0:1])
                            rstd = sbC.tile([1, TK], bf16, name=f"rstd{l}{tag2}", tag="rstd")
                            nc.scalar.activation(rstd[:], lnv[:], AF.Exp, scale=-0.5)
                            mr = sbC.tile([1, TK], bf16, name=f"mr{l}{tag2}", tag="mr")
                            nc.vector.tensor_mul(mr[:], m_sb[:], rstd[:])
                            psrb = psS.tile([128, TK], f32, name=f"rb{l}{tag2}", tag="rb")
                            psmb = psS.tile([128, TK], f32, name=f"mb{l}{tag2}", tag="mb")
                            mm(psrb[:], t_orow[:], rstd[:], True, True)
                            mm(psmb[:], t_orow[:], mr[:], True, True)
                            xn = sbC.tile([128, PT, TK], bf16, name=f"xn{l}{tag2}", tag=f"xn{TK}")
                            for p in range(PT):
                                tmp = sbC.tile([128, TK], bf16, name=f"lt{l}{tag2}{p}", tag=f"lt{TK}")
                                nc.vector.tensor_mul(tmp[:], u[:, p, :], psrb[:])
                                nc.vector.tensor_sub(xn[:, p, :], tmp[:], psmb[:])
                            return xn

                        def layer_tail(ctx_of, stream_of, TK, tg):
                            u = sbC.tile([128, PT, TK], bf16, name=f"u{l}{tg}", tag=f"u{TK}")
                            for mt in range(PT):
                                ps = psO.tile([128, TK], f32, name=f"o{l}{tg}{mt}", tag="oPS")
                                for ks in range(PT):
                                    mm(ps[:], t_wo[l][:, ks, 128 * mt:128 * (mt + 1)],
                                       ctx_of(ks), ks == 0, False)
                                for ks in range(PT):
                                    mm(ps[:], t_wo[l][:, PT + ks, 128 * mt:128 * (mt + 1)],
                                       stream_of(ks), False, ks == PT - 1)
                                nc.scalar.activation(u[:, mt, :], ps[:], AF.Identity,
                                                     bias=t_bo[l][:, mt, :])
                            xn = self_ln(u, TK, f"{tg}a")
                            if debug and l == 0 and tg == "d0":
                                nc.sync.dma_start(dbg["dbg_u"][:], u[:])
                                nc.sync.dma_start(dbg["dbg_xn1"][:], xn[:])
                            fa = sbC.tile([128, FT, TK], bf16, name=f"fa{l}{tg}", tag=f"fa{TK}")
                            for mt in range(FT):
                                ps = psO.tile([128, TK], f32, name=f"f1{l}{tg}{mt}", tag="oPS")
                                for ks in range(PT):
                                    mm(ps[:], t_w1[l][:, ks, 128 * mt:128 * (mt + 1)],
                                       xn[:, ks, :], ks == 0, ks == PT - 1)
                                nc.scalar.activation(fa[:, mt, :], ps[:], AF.Relu,
                                                     bias=t_c1[l][:, mt, :])
                            u2 = sbC.tile([128, PT, TK], bf16, name=f"u2{l}{tg}", tag=f"u{TK}")
                            for mt in range(PT):
                                ps = psO.tile([128, TK], f32, name=f"f2{l}{tg}{mt}", tag="oPS")
                                for ks in range(FT):
                                    mm(ps[:], t_w2[l][:, ks, 128 * mt:128 * (mt + 1)],
                                       fa[:, ks, :], ks == 0, False)
                                for ks in range(PT):
                                    mm(ps[:], t_w2[l][:, FT + ks, 128 * mt:128 * (mt + 1)],
                                       xn[:, ks, :], False, ks == PT - 1)
                                nc.scalar.activation(u2[:, mt, :], ps[:], AF.Identity,
                                                     bias=t_c2[l][:, mt, :])
                            xn2 = self_ln(u2, TK, f"{tg}b")
                            for p in range(PT):
                                nc.vector.tensor_copy(stream_of(p), xn2[:, p, :])

                        for j in range(4):
                            layer_tail(lambda ks, j=j: ctxD[j][:, ks, :],
                                       lambda p, j=j: sD[j][:, p, :], D, f"d{j}")
                        layer_tail(lambda ks: ctxQ[:, ks, :],
                                   lambda p: sQ[:, p, :], QP, "q")
                if debug and l == 0:
                    nc.sync.dma_start(dbg["dbg_s1"][:], sD[0][:])

            if debug:
                nc.sync.dma_start(dbg["dbg_sq"][:], sQ[:])

            # ---------------- tail: mixing, norms, stopword, pooling --------
            with ExitStack() as phD:
                psH = phD.enter_context(tc.tile_pool(name="psH", bufs=1, space="PSUM"))
                ps_swq = psH.tile([128, D], f32, name="ps_swq")
                psD = phD.enter_context(tc.tile_pool(name="psD", bufs=1, space="PSUM"))
                sbD = phD.enter_context(tc.tile_pool(name="sbD", bufs=2))
                dnD = [sbD.tile([128, PT, D], bf16, name=f"dn{j}", tag=f"dn{j}") for j in range(4)]
                qnQ = sbD.tile([128, PT, QP], bf16, name="qnQ", tag="qnQ")

                def normalize(src_of, hm_of, TK, tg, out_tile, eps):
                    mix = sbD.tile([128, PT, TK], bf16, name=f"mx{tg}", tag=f"mx{TK}")
                    for p in range(PT):
                        nc.vector.scalar_tensor_tensor(
                            out=mix[:, p, :], in0=src_of(p), scalar=mixer,
                            in1=hm_of(p), op0=AL.mult, op1=AL.add)
                    msq = sbD.tile([128, PT, TK], bf16, name=f"msq{tg}", tag=f"ms{TK}")
                    for p in range(PT):
                        nc.vector.tensor_mul(msq[:, p, :], mix[:, p, :], mix[:, p, :])
                    nsum = psD.tile([1, TK], f32, name=f"ns{tg}", tag="ns")
                    for p in range(PT):
                        mm(nsum[:], t_ocol[:], msq[:, p, :], p == 0, p == PT - 1)
                    lnn = sbD.tile([1, TK], f32, name=f"lnn{tg}", tag="lnn")
                    nc.scalar.activation(lnn[:], nsum[:], AF.Ln,
                                         bias=t_eps[0:1, 1:2])
                    rn = sbD.tile([1, TK], bf16, name=f"rn{tg}", tag="rn")
                    nc.scalar.activation(rn[:], lnn[:], AF.Exp, scale=-0.5)
                    psb = psD.tile([128, TK], f32, name=f"nb{tg}", tag="nb")
                    mm(psb[:], t_orow[:], rn[:], True, True)
                    for p in range(PT):
                        nc.vector.tensor_mul(out_tile[:, p, :], mix[:, p, :], psb[:])

                hmD = [sbD.tile([128, PT, D], bf16, name=f"hm{j}", tag=f"hmD{j}") for j in range(4)]
                hmQ = sbD.tile([128, PT, QP], bf16, name="hmQ", tag="hmQ")
                for j in range(4):
                    for p in range(PT):
                        nc.vector.tensor_scalar(
                            out=hmD[j][:, p, :], in0=sD[j][:, p, :],
                            scalar1=t_hs[:, p, :], scalar2=t_hb[:, p, :],
                            op0=AL.mult, op1=AL.add)
                for p in range(PT):
                    nc.vector.tensor_scalar(
                        out=hmQ[:, p, :], in0=sQ[:, p, :],
                        scalar1=t_hs[:, p, :], scalar2=t_hb[:, p, :],
                        op0=AL.mult, op1=AL.add)
                for j in range(4):
                    normalize(lambda p, j=j: embd[j][:, p, :],
                              lambda p, j=j: hmD[j][:, p, :], D, f"d{j}",
                              dnD[j], 1e-26)
                normalize(lambda p: embqP[:, p, :], lambda p: hmQ[:, p, :],
                          QP, "q", qnQ, 1e-26)

                for j in range(4):
                    embh = sbD.tile([128, PT, D], bf16, name=f"embh{j}", tag="embh")
                    for p in range(PT):
                        nc.vector.tensor_copy(embh[:, p, :], embd[j][:, p, :])
                    psz = psD.tile([128, D], f32, name=f"z{j}", tag="z")
                    for ks in range(PT):
                        mm(psz[:], t_wz[:, ks, :], embh[:, ks, :], ks == 0, False)
                    for ks in range(PT):
                        mm(psz[:], t_wz[:, PT + ks, :], sD[j][:, ks, :],
                           False, ks == PT - 1)
                    texp = sbD.tile([128, D], f32, name=f"texp{j}", tag="texp")
                    nc.scalar.activation(texp[:], psz[:], AF.Exp, bias=t_bz2[:],
                                         scale=2.0)
                    b1t = sbD.tile([128, D], f32, name=f"b1t{j}", tag="b1t")
                    nc.vector.tensor_scalar_add(b1t[:], texp[:], 1.0)
                    rec = sbD.tile([128, D], f32, name=f"recz{j}", tag="recz")
                    nc.vector.reciprocal_approx_fast(out=rec[:], in_=b1t[:])
                    tnh = sbD.tile([128, D], bf16, name=f"tnh{j}", tag="tnh")
                    nc.vector.tensor_scalar(out=tnh[:], in0=rec[:], scalar1=-2.0,
                                            scalar2=1.0, op0=AL.mult, op1=AL.add)
                    nc.sync.dma_start(tnh[100:101, :], w_dlogm_row[j])
                    mm(ps_swq[32 * j:32 * (j + 1), :], t_sw2[:], tnh[:],
                       True, True, tp=(0, 32 * j))
                    if debug and j == 0:
                        nc.sync.dma_start(dbg["dbg_tnh"][:], tnh[:])
                swb = sbD.tile([128, D], f32, name="swb")
                nc.scalar.activation(swb[:], ps_swq[:], AF.Relu, bias=t_bsw[:])
                for j in range(4):
                    nc.sync.dma_start(w_sw[j:j + 1], swb[32 * j:32 * j + 1, :])
                if debug:
                    nc.sync.dma_start(dbg["dbg_dn"][:], dnD[0][:])
                    nc.sync.dma_start(dbg["dbg_qn"][:], qnQ[:])
                    nc.sync.dma_start(dbg["dbg_swb"][:], swb[:])

                ps_cos = psD.tile([128, D], f32, name="ps_cos", tag="cos")
                for j in range(4):
                    for ks in range(PT):
                        mm(ps_cos[32 * j:32 * (j + 1), :],
                           qnQ[:, ks, 32 * j:32 * (j + 1)], dnD[j][:, ks, :],
                           ks == 0, ks == PT - 1, tp=(0, 32 * j))
                if debug:
                    ccp = sbD.tile([128, D], f32, name="dbgcos")
                    nc.vector.tensor_copy(ccp[:], ps_cos[:])
                    nc.sync.dma_start(dbg["dbg_cos"][:], ccp[:])
                pkq = sbD.tile([128, K], f32, name="pkq")
                swh = sbD.tile([128, D], bf16, name="swh")
                nc.vector.tensor_copy(swh[:], swb[:])
                for k in range(K):
                    ps_sq = psD.tile([128, D], f32, name=f"sq{k}", tag="sqp")
                    nc.scalar.activation(ps_sq[:], ps_cos[:], AF.Square,
                                         bias=t_negmu[:, k:k + 1])
                    ek = sbD.tile([128, D], bf16, name=f"ek{k}", tag="ek")
                    nc.scalar.activation(ek[:], ps_sq[:], AF.Exp, scale=aks[k])
                    prod = sbD.tile([128, D], f32, name=f"prod{k}", tag="prod")
                    nc.vector.scalar_tensor_tensor(
                        out=prod[:], in0=ek[:], scalar=alph[k], in1=swh[:],
                        op0=AL.mult, op1=AL.mult)
                    nc.vector.tensor_reduce(pkq[:, k:k + 1], prod[:],
                                            axis=mybir.AxisListType.X, op=AL.add)
                if debug:
                    nc.sync.dma_start(dbg["dbg_pkq"][:], pkq[:])
                pkc = sbD.tile([128, K], f32, name="pkc")
                nc.vector.tensor_scalar_max(pkc[:], pkq[:], 1e-10)
                lnp = sbD.tile([128, K], bf16, name="lnp")
                nc.scalar.activation(lnp[:], pkc[:], AF.Ln)
                ps_pk = psD.tile([4, K], f32, name="ps_pk", tag="pk")
                mm(ps_pk[:], t_qbd[:], lnp[:], True, True)
                scscr = sbD.tile([4, K], f32, name="scscr")
                nc.vector.tensor_mul(scscr[:], ps_pk[:], t_kw4[:])
                sc4 = sbD.tile([4, 1], f32, name="sc4")
                nc.vector.tensor_reduce(sc4[:], scscr[:],
                                        axis=mybir.AxisListType.X, op=AL.add)
                nc.sync.dma_start(w_score[:], sc4[:, 0])

    nc.finalize()
    return nc


# -------------------------------------------------------------------- runner
def _get_built(inp, debug=False):
    key = "dbg" if debug else "rel"
    if key not in _CACHE:
        consts = _prep_consts(inp)
        nc = _build(consts, debug=debug)
        _CACHE[key] = (nc, consts)
    return _CACHE[key]


def make_in_maps(inputs, consts):
    const_map = {k: v for k, v in consts.items() if not k.startswith("_")}
    in_maps = []
    for core in range(NCORES):
        m = dict(const_map)
        m.update(_prep_percore(inputs, core))
        in_maps.append(m)
    return in_maps


def kernel(**inputs):
    from concourse.bass_utils import run_bass_kernel_spmd

    nc, consts = _get_built(inputs, debug=False)
    in_maps = make_in_maps(inputs, consts)
    res = run_bass_kernel_spmd(nc, in_maps, list(range(NCORES)))
    score = np.concatenate([res.results[c]["score_out"] for c in range(NCORES)], 0)
    sw = np.concatenate([res.results[c]["sw_out"] for c in range(NCORES)], 0)
    return score.astype(np.float32), sw.reshape(B, 1, D).astype(np.float32)


# revision 14
# speedup vs baseline: 1.0140x; 1.0140x over previous
"""TK (Transformer-Kernel) ranking model on 8 Trainium2 NeuronCores.

Data parallel: batch 256 -> 8 cores x 32. Each core runs one fused
Bass/Tile kernel over its 32 batch items (processed as 8 quads of 4,
hardware For_i loop over quads).

Layout: feature-major activations [E(partitions), tokens(free)].
Attention is computed transposed (scores [k_tok(p), q_tok(f)]) so the key
mask + 1/sqrt(dh) fold into a single ACT exp; the softmax denominator and
context are matmuls against the exp matrix; the 1/den lands in the PSUM
evict. All matmul operands are bf16 (fp32 accumulate). LayerNorm affines,
biases and mixers are folded into adjacent matmul weights on the host.
"""

import math
import numpy as np
import ml_dtypes

BF = ml_dtypes.bfloat16

NCORES = 8
B, Q, D, E, FF, L, K, NH = 256, 32, 512, 256, 1024, 2, 11, 8
BC = B // NCORES          # 32 per core
NQUAD = BC // 4
DH = E // NH              # 32
PT = E // 128             # 2
DT = D // 128             # 4
FT = FF // 128            # 8
QP = 4 * Q                # packed query free dim (4 batch x 32 tokens)
LN_EPS = 1e-5

_CACHE = {}


# ----------------------------------------------------------------- host prep
def _pos_enc(dim, length):
    ts_ = np.arange(length, dtype=np.float32)
    n = dim // 2
    inv = np.exp(np.arange(n, dtype=np.float32) * (-math.log(10000.0) / (n - 1)))
    st = ts_[:, None] * inv[None, :]
    return np.concatenate([np.sin(st), np.cos(st)], axis=1).astype(np.float32)


def _fm(x):
    """[.., T, E] -> feature-major [.., PT, 128, T]"""
    xt = np.swapaxes(x, -1, -2)
    shp = xt.shape[:-2] + (PT, 128, xt.shape[-1])
    return np.ascontiguousarray(xt.reshape(shp))


def _prep_consts(inp):
    f32 = np.float32
    ipw = np.asarray(inp["in_proj_w"], f32)
    ipb = np.asarray(inp["in_proj_b"], f32)
    ow = np.asarray(inp["out_w"], f32)
    ob = np.asarray(inp["out_b"], f32)
    g1 = np.asarray(inp["ln1_g"], f32); b1 = np.asarray(inp["ln1_b"], f32)
    g2 = np.asarray(inp["ln2_g"], f32); b2 = np.asarray(inp["ln2_b"], f32)
    f1w = np.asarray(inp["ff1_w"], f32); f1b = np.asarray(inp["ff1_b"], f32)
    f2w = np.asarray(inp["ff2_w"], f32); f2b = np.asarray(inp["ff2_b"], f32)
    mixer = float(np.asarray(inp["mixer"]).reshape(-1)[0])
    ms = float(np.asarray(inp["mixer_stop"]).reshape(-1)[0])
    s1w = np.asarray(inp["sw1_w"], f32); s1b = np.asarray(inp["sw1_b"], f32)
    s2w = np.asarray(inp["sw2_w"], f32); s2b = np.asarray(inp["sw2_b"], f32)
    mu = np.asarray(inp["mu"], f32).reshape(-1)
    sigma = np.asarray(inp["sigma"], f32).reshape(-1)
    kw = np.asarray(inp["kernel_w"], f32).reshape(-1)
    alpha = np.asarray(inp["alpha"], f32).reshape(-1)

    c = {}
    pos = _pos_enc(E, max(Q, D))
    c["pos"] = _fm(pos[None])[0]                                   # [2,128,512]
    c["posq"] = np.ascontiguousarray(
        np.tile(c["pos"][:, :, :Q], (1, 1, 4)))                    # [2,128,128]

    gs = np.ones(E, f32)
    bs = np.zeros(E, f32)
    for l in range(L):
        Wq = ipw[l][:E] * gs[None, :]
        Wk = ipw[l][E:2 * E] * gs[None, :]
        Wv = ipw[l][2 * E:] * gs[None, :]
        bqv = ipb[l][:E] + ipw[l][:E] @ bs
        bv = ipb[l][2 * E:] + ipw[l][2 * E:] @ bs
        c[f"wqk{l}"] = np.ascontiguousarray(
            np.concatenate([Wq.T, Wk.T], 1).reshape(PT, 128, 2 * E)).astype(BF)
        c[f"bq{l}"] = bqv.reshape(PT, 128, 1).astype(f32)
        c[f"wv{l}"] = np.ascontiguousarray(Wv.T.reshape(PT, 128, E)).astype(BF)
        woa = np.concatenate([ow[l].T, np.diag(gs)], 0)            # [512,256]
        c[f"wo{l}"] = np.ascontiguousarray(woa.reshape(2 * PT, 128, E)).astype(BF)
        c[f"bo{l}"] = (ob[l] + bs + ow[l] @ bv).reshape(PT, 128, 1).astype(f32)
        W1 = f1w[l] * g1[l][None, :]
        c[f"w1{l}"] = np.ascontiguousarray(W1.T.reshape(PT, 128, FF)).astype(BF)
        c[f"c1{l}"] = (f1b[l] + f1w[l] @ b1[l]).reshape(FT, 128, 1).astype(f32)
        w2a = np.concatenate([f2w[l].T, np.diag(g1[l])], 0)        # [1280,256]
        c[f"w2{l}"] = np.ascontiguousarray(w2a.reshape(FT + PT, 128, E)).astype(BF)
        c[f"c2{l}"] = (f2b[l] + b1[l]).reshape(PT, 128, 1).astype(f32)
        gs, bs = g2[l], b2[l]

    c["hscale"] = ((1 - mixer) * gs).reshape(PT, 128, 1).astype(f32)
    c["hbias"] = ((1 - mixer) * bs).reshape(PT, 128, 1).astype(f32)
    wza = np.concatenate([(ms * s1w).T, ((1 - ms) * s1w * gs[None, :]).T], 0)
    wz = np.zeros((2 * PT, 128, 128), f32)
    wz[:, :, :100] = wza.reshape(2 * PT, 128, 100)
    c["wz"] = wz.astype(BF)
    bz2 = np.zeros((128, 1), f32)
    bz2[:100, 0] = 2.0 * (s1b + (1 - ms) * (s1w @ bs))
    c["bz2"] = bz2
    s2rep = np.zeros((128, 32), f32)
    s2rep[:100, :] = s2w.reshape(100, 1)
    s2rep[100, :] = 1.0
    c["sw2rep"] = s2rep.astype(BF)
    c["bsw"] = np.full((128, 1), s2b[0], f32)

    c["negmu"] = np.tile(-mu[None, :], (128, 1)).astype(f32)
    c["kw4"] = np.tile(kw[None, :], (4, 1)).astype(f32)
    c["ones_scl"] = np.full((128, 1), 1.0 / E, f32).astype(BF)
    c["ones_col"] = np.ones((128, 1), f32).astype(BF)
    c["ones32"] = np.ones((128, 32), f32).astype(BF)
    c["ones_row"] = np.ones((1, 128), f32).astype(BF)
    c["epsc"] = np.array([[1e-5, 1e-26]], f32).repeat(128, 0)  # [128,2]
    id4 = np.zeros((32, 128), f32)
    for a in range(4):
        id4[np.arange(32), 32 * a + np.arange(32)] = 1.0
    c["id4"] = id4.astype(BF)

    c["_mixer"] = mixer
    c["_aks"] = (-1.0 / (2.0 * sigma ** 2)).astype(f32)
    c["_alpha"] = alpha.astype(f32)
    return c


def _prep_percore(inp, core):
    f32 = np.float32
    sl = slice(core * BC, (core + 1) * BC)
    qe = np.asarray(inp["query_embeddings"], f32)[sl]
    de = np.asarray(inp["document_embeddings"], f32)[sl]
    qm = np.asarray(inp["query_mask"], f32)[sl]
    dm = np.asarray(inp["document_mask"], f32)[sl]
    d = {}
    d["demb"] = _fm(de)                                            # [BC,2,128,512]
    d["qemb"] = _fm(qe)                                            # [BC,2,128,32]
    d["dlogm"] = np.ascontiguousarray(
        ((dm - 1.0) * 1e9).reshape(BC, DT, 128, 1)).astype(f32)
    d["dlogm_row"] = ((dm - 1.0) * 1e9).reshape(BC, 1, D).astype(BF)
    d["qlogm4"] = np.ascontiguousarray(
        np.tile((qm - 1.0) * 1e9, (1, 4)).reshape(BC, 128, 1)).astype(f32)
    qbd = np.zeros((NQUAD, 128, 4), f32)
    for g in range(NQUAD):
        for j in range(4):
            qbd[g, 32 * j:32 * (j + 1), j] = qm[4 * g + j]
    d["qmask_bd"] = np.repeat(qbd, 4, axis=0).astype(BF)           # [BC,128,4]
    return d


# ------------------------------------------------------------------- builder
def _pin_act_table():
    """Make the ACT-table chooser use only natural_log_exp_and_others
    (it contains every function this kernel uses: Exp, Ln, Square,
    Identity, Relu, Copy). The greedy per-function chooser otherwise
    alternates exp_and_others <-> natural_log, inserting ~416 table
    loads (~0.5ms + drains). Names/indices are preserved so the emitted
    act_func_set_id stays valid."""
    import concourse.bacc as bacc
    import concourse.hw_specs as hw_specs
    if getattr(bacc, "_act_tables_pinned", False):
        return
    real = hw_specs.get_activation_tables

    def pinned(arch):
        t = real(arch)
        return {name: (fns if name == "natural_log_exp_and_others" else set())
                for name, fns in t.items()}

    bacc.get_activation_tables = pinned
    bacc._act_tables_pinned = True


def _build(consts, debug=False):
    import concourse.bacc as bacc
    import concourse.mybir as mybir
    from concourse.bass import ds
    from concourse.tile import TileContext
    from contextlib import ExitStack

    _pin_act_table()

    f32 = mybir.dt.float32
    bf16 = mybir.dt.bfloat16
    AF = mybir.ActivationFunctionType
    AL = mybir.AluOpType

    mixer = consts["_mixer"]
    aks = [float(a) for a in consts["_aks"]]
    alph = [float(a) for a in consts["_alpha"]]
    SCL = 1.0 / math.sqrt(DH)

    nc = bacc.Bacc("TRN2", target_bir_lowering=False, debug=False)

    def din(name, shape, dt=f32):
        return nc.dram_tensor(name, list(shape), dt, kind="ExternalInput")

    demb = din("demb", (BC, PT, 128, D))
    qemb = din("qemb", (BC, PT, 128, Q))
    dlogm = din("dlogm", (BC, DT, 128, 1))
    dlogm_row = din("dlogm_row", (BC, 1, D), bf16)
    qlogm4 = din("qlogm4", (BC, 128, 1))
    qmask_bd = din("qmask_bd", (BC, 128, 4), bf16)
    pos = din("pos", (PT, 128, D))
    posq = din("posq", (PT, 128, QP))
    wqk = [din(f"wqk{l}", (PT, 128, 2 * E), bf16) for l in range(L)]
    bq = [din(f"bq{l}", (PT, 128, 1)) for l in range(L)]
    wv = [din(f"wv{l}", (PT, 128, E), bf16) for l in range(L)]
    wo = [din(f"wo{l}", (2 * PT, 128, E), bf16) for l in range(L)]
    bo = [din(f"bo{l}", (PT, 128, 1)) for l in range(L)]
    w1 = [din(f"w1{l}", (PT, 128, FF), bf16) for l in range(L)]
    c1 = [din(f"c1{l}", (FT, 128, 1)) for l in range(L)]
    w2 = [din(f"w2{l}", (FT + PT, 128, E), bf16) for l in range(L)]
    c2 = [din(f"c2{l}", (PT, 128, 1)) for l in range(L)]
    hscale = din("hscale", (PT, 128, 1))
    hbias = din("hbias", (PT, 128, 1))
    wz = din("wz", (2 * PT, 128, 128), bf16)
    bz2 = din("bz2", (128, 1))
    sw2rep = din("sw2rep", (128, 32), bf16)
    bsw = din("bsw", (128, 1))
    negmu = din("negmu", (128, K))
    kw4 = din("kw4", (4, K))
    ones_scl = din("ones_scl", (128, 1), bf16)
    ones_col = din("ones_col", (128, 1), bf16)
    ones32 = din("ones32", (128, 32), bf16)
    ones_row = din("ones_row", (1, 128), bf16)
    id4 = din("id4", (32, 128), bf16)
    epsc = din("epsc", (128, 2))

    score_out = nc.dram_tensor("score_out", [BC], f32, kind="ExternalOutput")
    sw_out = nc.dram_tensor("sw_out", [BC, D], f32, kind="ExternalOutput")
    dbg = {}
    if debug:
        def dout(name, shape, dt=bf16):
            dbg[name] = nc.dram_tensor(name, list(shape), dt, kind="ExternalOutput")
        dout("dbg_s0", (128, PT, D))
        dout("dbg_u", (128, PT, D))
        dout("dbg_xn1", (128, PT, D))
        dout("dbg_s1", (128, PT, D))
        dout("dbg_den", (128, D), f32)
        dout("dbg_ctx", (128, PT, D))
        dout("dbg_dn", (128, PT, D))
        dout("dbg_qn", (128, PT, QP))
        dout("dbg_cos", (128, D), f32)
        dout("dbg_tnh", (128, D))
        dout("dbg_swb", (128, D), f32)
        dout("dbg_pkq", (128, K), f32)
        dout("dbg_sq", (128, PT, QP))

    with TileContext(nc) as tc, ExitStack() as top:
        wp = top.enter_context(tc.tile_pool(name="wpool", bufs=1))

        def ld2(apdram, dt, name, n, x):
            t = wp.tile([128, n, x], dt, name=name)
            for p in range(n):
                nc.sync.dma_start(t[:, p, :], apdram[p])
            return t

        def ld1(apdram, shape, dt, name):
            t = wp.tile(list(shape), dt, name=name)
            nc.sync.dma_start(t[:], apdram[:])
            return t

        t_pos = ld2(pos, f32, "t_pos", PT, D)
        t_posq = ld2(posq, f32, "t_posq", PT, QP)
        t_wqk = [ld2(wqk[l], bf16, f"t_wqk{l}", PT, 2 * E) for l in range(L)]
        t_bq = [ld2(bq[l], f32, f"t_bq{l}", PT, 1) for l in range(L)]
        t_wv = [ld2(wv[l], bf16, f"t_wv{l}", PT, E) for l in range(L)]
        t_wo = [ld2(wo[l], bf16, f"t_wo{l}", 2 * PT, E) for l in range(L)]
        t_bo = [ld2(bo[l], f32, f"t_bo{l}", PT, 1) for l in range(L)]
        t_w1 = [ld2(w1[l], bf16, f"t_w1{l}", PT, FF) for l in range(L)]
        t_c1 = [ld2(c1[l], f32, f"t_c1{l}", FT, 1) for l in range(L)]
        t_w2 = [ld2(w2[l], bf16, f"t_w2{l}", FT + PT, E) for l in range(L)]
        t_c2 = [ld2(c2[l], f32, f"t_c2{l}", PT, 1) for l in range(L)]
        t_hs = ld2(hscale, f32, "t_hs", PT, 1)
        t_hb = ld2(hbias, f32, "t_hb", PT, 1)
        t_wz = ld2(wz, bf16, "t_wz", 2 * PT, 128)
        t_bz2 = ld1(bz2, (128, 1), f32, "t_bz2")
        t_sw2 = ld1(sw2rep, (128, 32), bf16, "t_sw2")
        t_bsw = ld1(bsw, (128, 1), f32, "t_bsw")
        t_negmu = ld1(negmu, (128, K), f32, "t_negmu")
        t_kw4 = ld1(kw4, (4, K), f32, "t_kw4")
        t_oscl = ld1(ones_scl, (128, 1), bf16, "t_oscl")
        t_ocol = ld1(ones_col, (128, 1), bf16, "t_ocol")
        t_o32 = ld1(ones32, (128, 32), bf16, "t_o32")
        t_orow = ld1(ones_row, (1, 128), bf16, "t_orow")
        t_id4 = ld1(id4, (32, 128), bf16, "t_id4")
        t_eps = ld1(epsc, (128, 2), f32, "t_eps")

        def mm(psum_ap, lhsT, rhs, start, stop, tp=None):
            nc.tensor.matmul(psum_ap, lhsT, rhs, start=start, stop=stop,
                             tile_position=tp)

        with tc.For_i(0, BC, 4) as g, ExitStack() as body:
            w_demb = demb[ds(g, 4)]
            w_qemb = qemb[ds(g, 4)]
            w_dlogm = dlogm[ds(g, 4)]
            w_dlogm_row = dlogm_row[ds(g, 4)]
            w_qlogm4 = qlogm4[ds(g, 4)]
            w_qbd = qmask_bd[ds(g, 4)]
            w_score = score_out[ds(g, 4)]
            w_sw = sw_out[ds(g, 4)]

            iop = body.enter_context(tc.tile_pool(name="io", bufs=1))
            embd, t_dlg, t_qlg = [], [], []
            embqP = iop.tile([128, PT, QP], f32, name="embqP")
            for j in range(4):
                t = iop.tile([128, PT, D], f32, name=f"embd{j}")
                for p in range(PT):
                    nc.sync.dma_start(t[:, p, :], w_demb[j, p])
                embd.append(t)
                for p in range(PT):
                    nc.sync.dma_start(embqP[:, p, 32 * j:32 * (j + 1)],
                                      w_qemb[j, p])
                t = iop.tile([128, DT], f32, name=f"dlg{j}")
                for dt_ in range(DT):
                    nc.sync.dma_start(t[:, dt_:dt_ + 1], w_dlogm[j, dt_])
                t_dlg.append(t)
                t = iop.tile([128, 1], f32, name=f"qlg{j}")
                nc.sync.dma_start(t[:], w_qlogm4[j])
                t_qlg.append(t)
            t_qbd = iop.tile([128, 4], bf16, name="t_qbd")
            nc.sync.dma_start(t_qbd[:], w_qbd[0])

            # streams (bf16, feature-major); query packed 4b along free
            sD = [iop.tile([128, PT, D], bf16, name=f"sD{j}") for j in range(4)]
            sQ = iop.tile([128, PT, QP], bf16, name="sQ")
            with tc.tile_pool(name="prep", bufs=2):
                for j in range(4):
                    for p in range(PT):
                        nc.vector.tensor_add(sD[j][:, p, :], embd[j][:, p, :],
                                             t_pos[:, p, :])
                for p in range(PT):
                    nc.vector.tensor_add(sQ[:, p, :], embqP[:, p, :],
                                         t_posq[:, p, :])
            if debug:
                nc.sync.dma_start(dbg["dbg_s0"][:], sD[0][:])

            for l in range(L):
                with ExitStack() as lay:
                    # ------------ qkv (doc + query) -------------------------
                    sbA = lay.enter_context(tc.tile_pool(name=f"sbA{l}", bufs=1))
                    qfD = [sbA.tile([128, PT, D], bf16, name=f"qfD{l}{j}") for j in range(4)]
                    kfD = [sbA.tile([128, PT, D], bf16, name=f"kfD{l}{j}") for j in range(4)]
                    vD = [sbA.tile([128, DT, E], bf16, name=f"vD{l}{j}") for j in range(4)]
                    qfQ = sbA.tile([128, PT, QP], bf16, name=f"qfQ{l}")
                    kfQ = sbA.tile([128, PT, QP], bf16, name=f"kfQ{l}")
                    vQr = sbA.tile([128, 4, E], bf16, name=f"vQr{l}")
                    with ExitStack() as phA:
                        qkp = phA.enter_context(
                            tc.tile_pool(name=f"qkps{l}", bufs=2, space="PSUM"))
                        psA = phA.enter_context(
                            tc.tile_pool(name=f"psA{l}", bufs=1, space="PSUM"))
                        for j in range(4):
                            for mt in range(4):
                                ps = qkp.tile([128, D], f32, name=f"qk{l}{j}{mt}", tag="qk")
                                for ks in range(PT):
                                    mm(ps[:], t_wqk[l][:, ks, 128 * mt:128 * (mt + 1)],
                                       sD[j][:, ks, :], ks == 0, ks == PT - 1)
                                if mt < PT:
                                    nc.scalar.activation(qfD[j][:, mt, :], ps[:],
                                                         AF.Identity,
                                                         bias=t_bq[l][:, mt, :])
                                else:
                                    nc.scalar.copy(kfD[j][:, mt - PT, :], ps[:])
                            for dt_ in range(DT):
                                ps = qkp.tile([128, E], f32, name=f"vv{l}{j}{dt_}", tag="vv")
                                for ks in range(PT):
                                    mm(ps[:], sD[j][:, ks, 128 * dt_:128 * (dt_ + 1)],
                                       t_wv[l][:, ks, :], ks == 0, ks == PT - 1)
                                nc.scalar.copy(vD[j][:, dt_, :], ps[:])
                        for mt in range(4):
                            ps = qkp.tile([128, QP], f32, name=f"qkq{l}{mt}", tag="vv")
                            for ks in range(PT):
                                mm(ps[:], t_wqk[l][:, ks, 128 * mt:128 * (mt + 1)],
                                   sQ[:, ks, :], ks == 0, ks == PT - 1)
                            if mt < PT:
                                nc.scalar.activation(qfQ[:, mt, :], ps[:], AF.Identity,
                                                     bias=t_bq[l][:, mt, :])
                            else:
                                nc.scalar.copy(kfQ[:, mt - PT, :], ps[:])
                        psvq = psA.tile([32, 4, E], f32, name=f"vq{l}", tag="vq")
                        for j in range(4):
                            for ks in range(PT):
                                mm(psvq[:, j, :], sQ[:, ks, 32 * j:32 * (j + 1)],
                                   t_wv[l][:, ks, :], ks == 0, ks == PT - 1)
                        vq_sb = sbA.tile([32, 4, E], bf16, name=f"vqsb{l}")
                        nc.scalar.copy(vq_sb[:], psvq[:])
                        for half in range(2):
                            psr = psA.tile([128, 2 * E], f32, name=f"vqr{l}{half}", tag="vqrp")
                            mm(psr[:], t_id4[:],
                               vq_sb[:].rearrange("p a e -> p (a e)")[:, 512 * half:512 * (half + 1)],
                               True, True)
                            nc.vector.tensor_copy(
                                vQr[:].rearrange("p a e -> p (a e)")[:, 512 * half:512 * (half + 1)],
                                psr[:])

                    # ------------ attention (doc per-b; query per-b tiny) ---
                    sbB = lay.enter_context(tc.tile_pool(name=f"sbB{l}", bufs=1))
                    ctxD = [sbB.tile([128, PT, D], bf16, name=f"ctxD{l}{j}") for j in range(4)]
                    ctxQ = sbB.tile([128, PT, QP], bf16, name=f"ctxQ{l}")
                    with ExitStack() as phB:
                        psB = phB.enter_context(
                            tc.tile_pool(name=f"psB{l}", bufs=1, space="PSUM"))
                        etp = phB.enter_context(tc.tile_pool(name=f"et{l}", bufs=3))
                        red = phB.enter_context(tc.tile_pool(name=f"red{l}", bufs=2))
                        for j in range(4):
                            psden = [psB.tile([128, D], f32, name=f"den{l}{j}{hq}", tag=f"den{hq}") for hq in range(2)]
                            psctx = [psB.tile([128, D], f32, name=f"ctx{l}{j}{hq}", tag=f"ctxp{hq}") for hq in range(2)]
                            for kt in range(DT):
                                for hq in range(2):
                                    pssc = psB.tile([128, 4, D], f32, name=f"sc{l}{j}{kt}{hq}", tag="sc4")
                                    for jj in range(4):
                                        mm(pssc[:, jj, :],
                                           kfD[j][32 * jj:32 * (jj + 1), hq, 128 * kt:128 * (kt + 1)],
                                           qfD[j][32 * jj:32 * (jj + 1), hq, :],
                                           True, True, tp=(32 * jj, 0))
                                    et = etp.tile([128, 4, D], bf16, name=f"et{l}{j}{kt}{hq}", tag="et")
                                    nc.scalar.activation(et[:], pssc[:], AF.Exp,
                                                         bias=t_dlg[j][:, kt:kt + 1],
                                                         scale=SCL)
                                    for jj in range(4):
                                        mm(psden[hq][32 * jj:32 * (jj + 1), :],
                                           t_o32[:], et[:, jj, :],
                                           kt == 0, kt == DT - 1, tp=(0, 32 * jj))
                                        mm(psctx[hq][32 * jj:32 * (jj + 1), :],
                                           vD[j][:, kt, 128 * hq + 32 * jj:128 * hq + 32 * (jj + 1)],
                                           et[:, jj, :],
                                           kt == 0, kt == DT - 1, tp=(0, 32 * jj))
                            for hq in range(2):
                                rec = red.tile([128, D], f32, name=f"recd{l}{j}{hq}", tag="recd")
                                nc.vector.reciprocal_approx_fast(out=rec[:], in_=psden[hq][:])
                                nc.vector.tensor_mul(ctxD[j][:, hq, :], psctx[hq][:], rec[:])
                            if debug and l == 0 and j == 0:
                                dcp = red.tile([128, D], f32, name="dbgden", tag="recd")
                                nc.vector.tensor_copy(dcp[:], psden[0][:])
                                nc.sync.dma_start(dbg["dbg_den"][:], dcp[:])
                        if debug and l == 0:
                            nc.sync.dma_start(dbg["dbg_ctx"][:], ctxD[0][:])

                        for j in range(4):
                            bsl = slice(32 * j, 32 * (j + 1))
                            psq_sc = [psB.tile([128, Q], f32, name=f"qsc{l}{j}{hq}", tag=f"den{hq}") for hq in range(2)]
                            psq_dc = [psB.tile([128, 2 * Q], f32, name=f"qdc{l}{j}{hq}", tag=f"ctxp{hq}") for hq in range(2)]
                            etq = [etp.tile([128, Q], bf16, name=f"etq{l}{j}{hq}", tag="etq") for hq in range(2)]
                            for hq in range(2):
                                for jj in range(4):
                                    rsl = slice(32 * jj, 32 * (jj + 1))
                                    mm(psq_sc[hq][rsl, :], kfQ[rsl, hq, bsl],
                                       qfQ[rsl, hq, bsl], True, True,
                                       tp=(32 * jj, 32 * jj))
                                nc.scalar.activation(etq[hq][:], psq_sc[hq][:], AF.Exp,
                                                     bias=t_qlg[j][:], scale=SCL)
                                for jj in range(4):
                                    rsl = slice(32 * jj, 32 * (jj + 1))
                                    mm(psq_dc[hq][rsl, 0:Q], t_o32[rsl, :],
                                       etq[hq][rsl, :], True, True,
                                       tp=(32 * jj, 32 * jj))
                                    mm(psq_dc[hq][rsl, Q:2 * Q],
                                       vQr[rsl, j, 128 * hq + 32 * jj:128 * hq + 32 * (jj + 1)],
                                       etq[hq][rsl, :], True, True,
                                       tp=(32 * jj, 32 * jj))
                                rec = red.tile([128, Q], f32, name=f"recq{l}{j}{hq}", tag="recq")
                                nc.vector.reciprocal_approx_fast(out=rec[:], in_=psq_dc[hq][:, 0:Q])
                                nc.vector.tensor_mul(ctxQ[:, hq, bsl],
                                                     psq_dc[hq][:, Q:2 * Q], rec[:])

                    # ------------ o-proj + LN1 + ff + LN2 -------------------
                    with ExitStack() as phC:
                        psO = phC.enter_context(tc.tile_pool(name=f"psO{l}", bufs=2, space="PSUM"))
                        psS = phC.enter_context(tc.tile_pool(name=f"psS{l}", bufs=1, space="PSUM"))
                        sbC = phC.enter_context(tc.tile_pool(name=f"sbC{l}", bufs=2))

                        def self_ln(u, TK, tag2):
                            usq = sbC.tile([128, PT, TK], bf16, name=f"usq{l}{tag2}", tag=f"usq{TK}")
                            for p in range(PT):
                                nc.vector.tensor_mul(usq[:, p, :], u[:, p, :], u[:, p, :])
                            st0 = psS.tile([1, TK], f32, name=f"st0{l}{tag2}", tag="st0")
                            st1 = psS.tile([1, TK], f32, name=f"st1{l}{tag2}", tag="st1")
                            for p in range(PT):
                                mm(st0[:], t_oscl[:], u[:, p, :], p == 0, p == PT - 1)
                            for p in range(PT):
                                mm(st1[:], t_oscl[:], usq[:, p, :], p == 0, p == PT - 1)
                            m_sb = sbC.tile([1, TK], f32, name=f"msb{l}{tag2}", tag="msb")
                            nc.vector.tensor_copy(m_sb[:], st0[:])
                            m2 = sbC.tile([1, TK], f32, name=f"m2{l}{tag2}", tag="m2")
                            nc.vector.tensor_mul(m2[:], m_sb[:], m_sb[:])
                            var = sbC.tile([1, TK], f32, name=f"var{l}{tag2}", tag="var")
                            nc.vector.tensor_sub(var[:], st1[:], m2[:])
                            lnv = sbC.tile([1, TK], f32, name=f"lnv{l}{tag2}", tag="lnv")
                            nc.scalar.activation(lnv[:], var[:], AF.Ln,
                                                 bias=t_eps[0:1, 0:1])
                            rstd = sbC.tile([1, TK], bf16, name=f"rstd{l}{tag2}", tag="rstd")
                            nc.scalar.activation(rstd[:], lnv[:], AF.Exp, scale=-0.5)
                            mr = sbC.tile([1, TK], bf16, name=f"mr{l}{tag2}", tag="mr")
                            nc.vector.tensor_mul(mr[:], m_sb[:], rstd[:])
                            psrb = psS.tile([128, TK], f32, name=f"rb{l}{tag2}", tag="rb")
                            psmb = psS.tile([128, TK], f32, name=f"mb{l}{tag2}", tag="mb")
                            mm(psrb[:], t_orow[:], rstd[:], True, True)
                            mm(psmb[:], t_orow[:], mr[:], True, True)
                            xn = sbC.tile([128, PT, TK], bf16, name=f"xn{l}{tag2}", tag=f"xn{TK}")
                            for p in range(PT):
                                tmp = sbC.tile([128, TK], bf16, name=f"lt{l}{tag2}{p}", tag=f"lt{TK}")
                                nc.vector.tensor_mul(tmp[:], u[:, p, :], psrb[:])
                                nc.vector.tensor_sub(xn[:, p, :], tmp[:], psmb[:])
                            return xn

                        def layer_tail(ctx_of, stream_of, TK, tg):
                            u = sbC.tile([128, PT, TK], bf16, name=f"u{l}{tg}", tag=f"u{TK}")
                            for mt in range(PT):
                                ps = psO.tile([128, TK], f32, name=f"o{l}{tg}{mt}", tag="oPS")
                                for ks in range(PT):
                                    mm(ps[:], t_wo[l][:, ks, 128 * mt:128 * (mt + 1)],
                                       ctx_of(ks), ks == 0, False)
                                for ks in range(PT):
                                    mm(ps[:], t_wo[l][:, PT + ks, 128 * mt:128 * (mt + 1)],
                                       stream_of(ks), False, ks == PT - 1)
                                nc.scalar.activation(u[:, mt, :], ps[:], AF.Identity,
                                                     bias=t_bo[l][:, mt, :])
                            xn = self_ln(u, TK, f"{tg}a")
                            if debug and l == 0 and tg == "d0":
                                nc.sync.dma_start(dbg["dbg_u"][:], u[:])
                                nc.sync.dma_start(dbg["dbg_xn1"][:], xn[:])
                            fa = sbC.tile([128, FT, TK], bf16, name=f"fa{l}{tg}", tag=f"fa{TK}")
                            for mt in range(FT):
                                ps = psO.tile([128, TK], f32, name=f"f1{l}{tg}{mt}", tag="oPS")
                                for ks in range(PT):
                                    mm(ps[:], t_w1[l][:, ks, 128 * mt:128 * (mt + 1)],
                                       xn[:, ks, :], ks == 0, ks == PT - 1)
                                nc.scalar.activation(fa[:, mt, :], ps[:], AF.Relu,
                                                     bias=t_c1[l][:, mt, :])
                            u2 = sbC.tile([128, PT, TK], bf16, name=f"u2{l}{tg}", tag=f"u{TK}")
                            for mt in range(PT):
                                ps = psO.tile([128, TK], f32, name=f"f2{l}{tg}{mt}", tag="oPS")
                                for ks in range(FT):
                                    mm(ps[:], t_w2[l][:, ks, 128 * mt:128 * (mt + 1)],
                                       fa[:, ks, :], ks == 0, False)
                                for ks in range(PT):
                                    mm(ps[:], t_w2[l][:, FT + ks, 128 * mt:128 * (mt + 1)],
                                       xn[:, ks, :], False, ks == PT - 1)
                                nc.scalar.activation(u2[:, mt, :], ps[:], AF.Identity,
                                                     bias=t_c2[l][:, mt, :])
                            xn2 = self_ln(u2, TK, f"{tg}b")
                            for p in range(PT):
                                nc.vector.tensor_copy(stream_of(p), xn2[:, p, :])

                        for j in range(4):
                            layer_tail(lambda ks, j=j: ctxD[j][:, ks, :],
                                       lambda p, j=j: sD[j][:, p, :], D, f"d{j}")
                        layer_tail(lambda ks: ctxQ[:, ks, :],
                                   lambda p: sQ[:, p, :], QP, "q")
                if debug and l == 0:
                    nc.sync.dma_start(dbg["dbg_s1"][:], sD[0][:])

            if debug:
                nc.sync.dma_start(dbg["dbg_sq"][:], sQ[:])

            # ---------------- tail: mixing, norms, stopword, pooling --------
            with ExitStack() as phD:
                psH = phD.enter_context(tc.tile_pool(name="psH", bufs=1, space="PSUM"))
                ps_swq = psH.tile([128, D], f32, name="ps_swq")
                psD = phD.enter_context(tc.tile_pool(name="psD", bufs=1, space="PSUM"))
                sbD = phD.enter_context(tc.tile_pool(name="sbD", bufs=2))
                dnD = [sbD.tile([128, PT, D], bf16, name=f"dn{j}", tag=f"dn{j}") for j in range(4)]
                qnQ = sbD.tile([128, PT, QP], bf16, name="qnQ", tag="qnQ")

                def normalize(src_of, hm_of, TK, tg, out_tile, eps):
                    mix = sbD.tile([128, PT, TK], bf16, name=f"mx{tg}", tag=f"mx{TK}")
                    for p in range(PT):
                        nc.vector.scalar_tensor_tensor(
                            out=mix[:, p, :], in0=src_of(p), scalar=mixer,
                            in1=hm_of(p), op0=AL.mult, op1=AL.add)
                    msq = sbD.tile([128, PT, TK], bf16, name=f"msq{tg}", tag=f"ms{TK}")
                    for p in range(PT):
                        nc.vector.tensor_mul(msq[:, p, :], mix[:, p, :], mix[:, p, :])
                    nsum = psD.tile([1, TK], f32, name=f"ns{tg}", tag="ns")
                    for p in range(PT):
                        mm(nsum[:], t_ocol[:], msq[:, p, :], p == 0, p == PT - 1)
                    lnn = sbD.tile([1, TK], f32, name=f"lnn{tg}", tag="lnn")
                    nc.scalar.activation(lnn[:], nsum[:], AF.Ln,
                                         bias=t_eps[0:1, 1:2])
                    rn = sbD.tile([1, TK], bf16, name=f"rn{tg}", tag="rn")
                    nc.scalar.activation(rn[:], lnn[:], AF.Exp, scale=-0.5)
                    psb = psD.tile([128, TK], f32, name=f"nb{tg}", tag="nb")
                    mm(psb[:], t_orow[:], rn[:], True, True)
                    for p in range(PT):
                        nc.vector.tensor_mul(out_tile[:, p, :], mix[:, p, :], psb[:])

                hmD = [sbD.tile([128, PT, D], bf16, name=f"hm{j}", tag=f"hmD{j}") for j in range(4)]
                hmQ = sbD.tile([128, PT, QP], bf16, name="hmQ", tag="hmQ")
                for j in range(4):
                    for p in range(PT):
                        nc.vector.tensor_scalar(
                            out=hmD[j][:, p, :], in0=sD[j][:, p, :],
                            scalar1=t_hs[:, p, :], scalar2=t_hb[:, p, :],
                            op0=AL.mult, op1=AL.add)
                for p in range(PT):
                    nc.vector.tensor_scalar(
                        out=hmQ[:, p, :], in0=sQ[:, p, :],
                        scalar1=t_hs[:, p, :], scalar2=t_hb[:, p, :],
                        op0=AL.mult, op1=AL.add)
                for j in range(4):
                    normalize(lambda p, j=j: embd[j][:, p, :],
                              lambda p, j=j: hmD[j][:, p, :], D, f"d{j}",
                              dnD[j], 1e-26)
                normalize(lambda p: embqP[:, p, :], lambda p: hmQ[:, p, :],
                          QP, "q", qnQ, 1e-26)

                for j in range(4):
                    embh = sbD.tile([128, PT, D], bf16, name=f"embh{j}", tag="embh")
                    for p in range(PT):
                        nc.vector.tensor_copy(embh[:, p, :], embd[j][:, p, :])
                    psz = psD.tile([128, D], f32, name=f"z{j}", tag="z")
                    for ks in range(PT):
                        mm(psz[:], t_wz[:, ks, :], embh[:, ks, :], ks == 0, False)
                    for ks in range(PT):
                        mm(psz[:], t_wz[:, PT + ks, :], sD[j][:, ks, :],
                           False, ks == PT - 1)
                    texp = sbD.tile([128, D], f32, name=f"texp{j}", tag="texp")
                    nc.scalar.activation(texp[:], psz[:], AF.Exp, bias=t_bz2[:],
                                         scale=2.0)
                    b1t = sbD.tile([128, D], f32, name=f"b1t{j}", tag="b1t")
                    nc.vector.tensor_scalar_add(b1t[:], texp[:], 1.0)
                    rec = sbD.tile([128, D], f32, name=f"recz{j}", tag="recz")
                    nc.vector.reciprocal_approx_fast(out=rec[:], in_=b1t[:])
                    tnh = sbD.tile([128, D], bf16, name=f"tnh{j}", tag="tnh")
                    nc.vector.tensor_scalar(out=tnh[:], in0=rec[:], scalar1=-2.0,
                                            scalar2=1.0, op0=AL.mult, op1=AL.add)
                    nc.sync.dma_start(tnh[100:101, :], w_dlogm_row[j])
                    mm(ps_swq[32 * j:32 * (j + 1), :], t_sw2[:], tnh[:],
                       True, True, tp=(0, 32 * j))
                    if debug and j == 0:
                        nc.sync.dma_start(dbg["dbg_tnh"][:], tnh[:])
                swb = sbD.tile([128, D], f32, name="swb")
                nc.scalar.activation(swb[:], ps_swq[:], AF.Relu, bias=t_bsw[:])
                for j in range(4):
                    nc.sync.dma_start(w_sw[j:j + 1], swb[32 * j:32 * j + 1, :])
                if debug:
                    nc.sync.dma_start(dbg["dbg_dn"][:], dnD[0][:])
                    nc.sync.dma_start(dbg["dbg_qn"][:], qnQ[:])
                    nc.sync.dma_start(dbg["dbg_swb"][:], swb[:])

                ps_cos = psD.tile([128, D], f32, name="ps_cos", tag="cos")
                for j in range(4):
                    for ks in range(PT):
                        mm(ps_cos[32 * j:32 * (j + 1), :],
                           qnQ[:, ks, 32 * j:32 * (j + 1)], dnD[j][:, ks, :],
                           ks == 0, ks == PT - 1, tp=(0, 32 * j))
                if debug:
                    ccp = sbD.tile([128, D], f32, name="dbgcos")
                    nc.vector.tensor_copy(ccp[:], ps_cos[:])
                    nc.sync.dma_start(dbg["dbg_cos"][:], ccp[:])
                pkq = sbD.tile([128, K], f32, name="pkq")
                swh = sbD.tile([128, D], bf16, name="swh")
                nc.vector.tensor_copy(swh[:], swb[:])
                for k in range(K):
                    ps_sq = psD.tile([128, D], f32, name=f"sq{k}", tag="sqp")
                    nc.scalar.activation(ps_sq[:], ps_cos[:], AF.Square,
                                         bias=t_negmu[:, k:k + 1])
                    ek = sbD.tile([128, D], bf16, name=f"ek{k}", tag="ek")
                    nc.scalar.activation(ek[:], ps_sq[:], AF.Exp, scale=aks[k])
                    prod = sbD.tile([128, D], f32, name=f"prod{k}", tag="prod")
                    nc.vector.scalar_tensor_tensor(
                        out=prod[:], in0=ek[:], scalar=alph[k], in1=swh[:],
                        op0=AL.mult, op1=AL.mult)
                    nc.vector.tensor_reduce(pkq[:, k:k + 1], prod[:],
                                            axis=mybir.AxisListType.X, op=AL.add)
                if debug:
                    nc.sync.dma_start(dbg["dbg_pkq"][:], pkq[:])
                pkc = sbD.tile([128, K], f32, name="pkc")
                nc.vector.tensor_scalar_max(pkc[:], pkq[:], 1e-10)
                lnp = sbD.tile([128, K], bf16, name="lnp")
                nc.scalar.activation(lnp[:], pkc[:], AF.Ln)
                ps_pk = psD.tile([4, K], f32, name="ps_pk", tag="pk")
                mm(ps_pk[:], t_qbd[:], lnp[:], True, True)
                scscr = sbD.tile([4, K], f32, name="scscr")
                nc.vector.tensor_mul(scscr[:], ps_pk[:], t_kw4[:])
                sc4 = sbD.tile([4, 1], f32, name="sc4")
                nc.vector.tensor_reduce(sc4[:], scscr[:],
                                        axis=mybir.AxisListType.X, op=AL.add)
                nc.sync.dma_start(w_score[:], sc4[:, 0])

    nc.finalize()
    return nc


# -------------------------------------------------------------------- runner
def _get_built(inp, debug=False):
    key = "dbg" if debug else "rel"
    if key not in _CACHE:
        consts = _prep_consts(inp)
        nc = _build(consts, debug=debug)
        _CACHE[key] = (nc, consts)
    return _CACHE[key]


def make_in_maps(inputs, consts):
    const_map = {k: v for k, v in consts.items() if not k.startswith("_")}
    in_maps = []
    for core in range(NCORES):
        m = dict(const_map)
        m.update(_prep_percore(inputs, core))
        in_maps.append(m)
    return in_maps


def kernel(**inputs):
    from concourse.bass_utils import run_bass_kernel_spmd

    nc, consts = _get_built(inputs, debug=False)
    in_maps = make_in_maps(inputs, consts)
    res = run_bass_kernel_spmd(nc, in_maps, list(range(NCORES)))
    score = np.concatenate([res.results[c]["score_out"] for c in range(NCORES)], 0)
    sw = np.concatenate([res.results[c]["sw_out"] for c in range(NCORES)], 0)
    return score.astype(np.float32), sw.reshape(B, 1, D).astype(np.float32)
